# revision 26
# baseline (speedup 1.0000x reference)
"""MixedQLinear Trainium2 kernel (v2: token-sharded).

Computation (per reference):
  x2 = x[0]                                  (M=4096, IN_F=4096) fp16
  int_x = x2[:, int_indices]                 (M, 3840)
  fp_x  = x2[:, fp_indices]                  (M, 256)
  per-token asym quant of int_x to int4:  scale=(mx-mn)/15, zero=mn
  q = round((int_x-zero)/scale) - 8          in [-8,7]
  out = scale*w_scale*(q @ w_int.T) + (zero+8*scale)*reduced_w + fp_x@fp_w.T + bias

Sharding: TOKENS are sharded 8 ways (512 tokens per core); every core
holds the FULL weight set (int4 weights as fp8 = 15 MiB) and computes
all 4096 out_features for its own tokens.  No collectives.  This makes
the per-token quantization work (stats, scale, round, transpose) 8x
smaller than out-feature sharding, and lets the int matmul run as long
fp8 DoubleRow instructions with ap_size 1024 so the PE stays busy and
ramps to its full clock.

Per core device flow (4 token tiles of 128):
  - stats: min tree (DVE) + max tree (GpSimd) -> scale, rs, bq, alpha
  - quantize: ACT y0 = x*rs+bq (f32, quarters), DVE RNE round via the
    fp32 magic constant -> qa f16 [128, 3968] (cols 3840/3841 = alpha/1)
  - two half DMA xbar transposes -> qt [128, 31, 128] k-major
  - ACT cast to fp8 -> q8 [128, 30, 128]
  - per 1024-col out chunk: 15 fp8 DoubleRow matmuls (q8 stationary,
    weights moving) -> p0; fp8-DR fp-path matmul + f16 extras matmul
    (alpha row x reduced_w, ones x bias) -> p1
  - combine: m = (p0*scale_t)*wscale (scalar_tensor_tensor), out = m+p1

Host side does layout only: column gather, int4 unpack, fp8 casts,
sharding, concat of per-core outputs along tokens.
"""

import os
import sys

import numpy as np

for _p in ("/opt/trn_rl_repo",):
    if _p not in sys.path and os.path.isdir(_p):
        sys.path.insert(0, _p)

TOKENS = 4096
IN_F = 4096
OUT_F = 4096
FP_F = 256
INT_F = IN_F - FP_F          # 3840
NCORES = 8
TPT = TOKENS // NCORES       # 512 own tokens per core
NT = TPT // 128              # 4 own token tiles
KE = INT_F // 128            # 30 k-groups
KE2 = KE // 2                # 15 DoubleRow steps
CH = 512                     # out-feature chunk (1 PSUM bank of f32)
NCH = OUT_F // CH            # 8 chunks
QW = 3968                    # qa width: 3840 q + alpha + ones + pad (31*128)
C_MAGIC = 12582912.0         # 1.5*2^23: fp32 add/sub forces RNE-to-integer

_PROGRAM = None
LAST_RESULTS = None


def _ensure_ntff_hook():
    """Install the axon NTFF profiling hook if the image's antenv lacks it.

    Best-effort: profiling only; compile/run work without it.
    """
    import contextlib
    import ctypes
    import types

    try:
        try:
            import antenv.axon_hooks as hooks_mod
        except ImportError:
            import antenv

            hooks_mod = types.ModuleType("antenv.axon_hooks")
            _holder = {}
            hooks_mod.set_axon_ntff_profile_hook = (
                lambda hook: _holder.__setitem__("hook", hook))
            hooks_mod.get_axon_ntff_profile_hook = (
                lambda: _holder.get("hook"))
            sys.modules["antenv.axon_hooks"] = hooks_mod
            antenv.axon_hooks = hooks_mod

        if hooks_mod.get_axon_ntff_profile_hook() is not None:
            return
        so_path = "/opt/axon/libaxon_pjrt.so"
        if not os.path.exists(so_path):
            return
        lib = ctypes.CDLL(so_path)
        if not hasattr(lib, "axon_start_nrt_profile"):
            return
        lib.axon_start_nrt_profile.argtypes = [
            ctypes.POINTER(ctypes.c_int64), ctypes.c_size_t]
        lib.axon_start_nrt_profile.restype = ctypes.c_int64
        lib.axon_stop_nrt_profile.argtypes = [ctypes.c_char_p]
        lib.axon_stop_nrt_profile.restype = ctypes.c_int64

        @contextlib.contextmanager
        def _hook(output_dir, device_ids):
            import jax

            jax.devices()
            if device_ids:
                ids = (ctypes.c_int64 * len(device_ids))(*device_ids)
                rc = lib.axon_start_nrt_profile(ids, len(device_ids))
            else:
                rc = lib.axon_start_nrt_profile(None, 0)
            if rc != 0:
                raise RuntimeError(f"axon_start_nrt_profile rc={rc}")
            try:
                yield
            finally:
                n = lib.axon_stop_nrt_profile(str(output_dir).encode())
                print(f"ntff profile: {n} file(s) written to {output_dir}")

        hooks_mod.set_axon_ntff_profile_hook(_hook)
    except Exception:
        pass


def _build_program():
    import concourse.mybir as mybir
    import concourse.tile as tile
    from concourse import bacc

    f16 = mybir.dt.float16
    f32 = mybir.dt.float32
    f8 = mybir.dt.float8e4
    Alu = mybir.AluOpType
    DR = mybir.MatmulPerfMode.DoubleRow

    nc = bacc.Bacc(None, target_bir_lowering=False)

    x_st = nc.dram_tensor("x_st", [TPT, INT_F], f16, kind="ExternalInput")
    # int4 weight values (exact in fp8e4m3), chunk-column-major k-major DR
    # layout: wq[c, p, e, r, o] = w_int[(2e+r)*128 + p, c*CH + o]
    wq_d = nc.dram_tensor("wq", [NCH, 128, KE2, 2, CH], f8,
                          kind="ExternalInput")
    fpx_d = nc.dram_tensor("fpx", [128, 2, TPT], f8, kind="ExternalInput")
    fpw_d = nc.dram_tensor("fpw", [128, 2, OUT_F], f8, kind="ExternalInput")
    wsb_d = nc.dram_tensor("wsb", [128, OUT_F], f16, kind="ExternalInput")
    rwb_d = nc.dram_tensor("rwb", [128, OUT_F], f16, kind="ExternalInput")
    out_d = nc.dram_tensor("out", [TPT, OUT_F], f16, kind="ExternalOutput")

    with tile.TileContext(nc) as tc:
        with tc.tile_pool(name="consts", bufs=1) as consts, \
             tc.tile_pool(name="xin", bufs=2) as xin, \
             tc.tile_pool(name="y0p", bufs=2) as y0p, \
             tc.tile_pool(name="qap", bufs=1) as qap, \
             tc.tile_pool(name="qtp", bufs=2) as qtp, \
             tc.tile_pool(name="qt8", bufs=2) as qt8, \
             tc.tile_pool(name="jnk", bufs=1) as jnk, \
             tc.tile_pool(name="stp", bufs=4) as stp, \
             tc.tile_pool(name="mp", bufs=1) as mp, \
             tc.tile_pool(name="outp", bufs=2) as outp, \
             tc.tile_pool(name="ps0", bufs=4, space="PSUM") as ps0, \
             tc.tile_pool(name="ps1", bufs=4, space="PSUM") as ps1:

            # Const loads on the scalar (ACT) HWDGE queue.  Order matters:
            # wq chunk-column c gates the consumer's chunk c, and the small
            # fp/scale tensors are needed from chunk 0 on, so they ride
            # between wq chunks 0 and 1.
            # All DMA transfers serialize on one DMA-engine resource, so
            # only global order matters: weight chunk-columns stream on the
            # scalar queue in consumption order, interleaved with the small
            # fp/scale tensors right when the first combines need them.
            wq_s = consts.tile([128, NCH, KE2, 2, CH], f8)
            nc.scalar.dma_start(out=wq_s[:, 0], in_=wq_d[0])
            fpw_s = consts.tile([128, 2, OUT_F], f8)
            nc.scalar.dma_start(out=fpw_s[:, :, :], in_=fpw_d[:, :, :])
            nc.scalar.dma_start(out=wq_s[:, 1], in_=wq_d[1])
            # Later weight chunks are time-staggered so their DMA requests
            # enter the (serializing) DMA-engine queue behind the tile-0/1
            # transposes and roughly at the PE's consumption rate.
            wsb_s = consts.tile([128, OUT_F], f16)
            rwb_s = consts.tile([128, OUT_F], f16)
            stagger = {2: 0.016, 3: 0.020, 4: 0.024, 5: 0.030,
                       6: 0.034, 7: 0.038}
            with tc.tile_wait_until(0.026):
                nc.scalar.dma_start(out=wsb_s[:, :], in_=wsb_d[:, :])
                nc.scalar.dma_start(out=rwb_s[:, :], in_=rwb_d[:, :])
            for c in range(2, NCH):
                with tc.tile_wait_until(stagger[c]):
                    nc.scalar.dma_start(out=wq_s[:, c], in_=wq_d[c])
            # own-token fp columns ride the gpsimd queue with the x tiles
            fpx_s = consts.tile([128, 2, TPT], f8)

            state = {}

            def prod_a(r):
                """x load, stats, quantize, round -> qa."""
                xt = xin.tile([128, INT_F], f16)
                nc.sync.dma_start(
                    out=xt[:, :], in_=x_st[r * 128:(r + 1) * 128, :])
                if r == 0:
                    nc.gpsimd.dma_start(out=fpx_s[:, :, :], in_=fpx_d[:, :, :])
                # min tree on DVE, max tree on GpSimd (parallel engines)
                mn = stp.tile([128, 1], f32, tag="mn")
                mx = stp.tile([128, 1], f32, tag="mx")
                a1 = jnk.tile([128, 1920], f16, tag="a1")
                nc.vector.tensor_tensor(
                    out=a1[:, :], in0=xt[:, :1920], in1=xt[:, 1920:],
                    op=Alu.min)
                nc.vector.tensor_reduce(
                    out=mn[:, :], in_=a1[:, :], axis=mybir.AxisListType.X,
                    op=Alu.min)
                # max tree reuses a1 (bufs=1 pool -> in-order WAR on DVE)
                a1b = jnk.tile([128, 1920], f16, tag="a1")
                nc.vector.tensor_tensor(
                    out=a1b[:, :], in0=xt[:, :1920], in1=xt[:, 1920:],
                    op=Alu.max)
                nc.vector.tensor_reduce(
                    out=mx[:, :], in_=a1b[:, :], axis=mybir.AxisListType.X,
                    op=Alu.max)
                # scale = max((mx-mn)/15, 1e-8); rs = 1/scale;
                # bq = -mn*rs - 8; alpha = mn + 8*scale
                d = stp.tile([128, 1], f32, tag="d")
                nc.vector.tensor_sub(d[:, :], mx[:, :], mn[:, :])
                sc = stp.tile([128, 1], f32, tag="sc")
                nc.vector.tensor_scalar(
                    out=sc[:, :], in0=d[:, :],
                    scalar1=1.0 / 15.0, scalar2=1e-8, op0=Alu.mult, op1=Alu.max)
                rs = stp.tile([128, 1], f32, tag="rs")
                nc.vector.reciprocal(rs[:, :], sc[:, :])
                tt = stp.tile([128, 1], f32, tag="tt")
                nc.vector.tensor_mul(tt[:, :], mn[:, :], rs[:, :])
                bq = stp.tile([128, 1], f32, tag="bq")
                nc.vector.tensor_scalar(
                    out=bq[:, :], in0=tt[:, :],
                    scalar1=-1.0, scalar2=-8.0, op0=Alu.mult, op1=Alu.add)
                t8 = stp.tile([128, 1], f32, tag="t8")
                nc.vector.tensor_scalar(
                    out=t8[:, :], in0=sc[:, :],
                    scalar1=8.0, scalar2=None, op0=Alu.mult)
                al = stp.tile([128, 1], f32, tag="al")
                nc.vector.tensor_add(al[:, :], t8[:, :], mn[:, :])
                qa = qap.tile([128, INT_F], f16)
                # quantize+round in eighths: ACT y0=x*rs+bq, DVE RNE round
                for h in range(8):
                    lo, hi = h * 480, (h + 1) * 480
                    y0 = y0p.tile([128, 480], f32, tag="y0")
                    nc.scalar.activation(
                        out=y0[:, :], in_=xt[:, lo:hi],
                        func=mybir.ActivationFunctionType.Identity,
                        bias=bq[:, :], scale=rs[:, :])
                    nc.vector.tensor_scalar(
                        out=qa[:, lo:hi], in0=y0[:, :], scalar1=C_MAGIC,
                        scalar2=-C_MAGIC, op0=Alu.add, op1=Alu.add)
                state[r] = (qa, sc, al)

            def prod_b(r):
                """Transpose qa -> qt (k-major), cast to fp8 -> q8."""
                qa, sc, al = state[r]
                qt = qtp.tile([128, KE, 128], f16)
                nc.sync.dma_start_transpose(
                    out=qt[:, :15, :], in_=qa[:, :1920])
                nc.sync.dma_start_transpose(
                    out=qt[:, 15:, :], in_=qa[:, 1920:])
                q8 = qt8.tile([128, KE, 128], f8)
                nc.scalar.copy(q8[:, :15, :], qt[:, :15, :])
                nc.scalar.copy(q8[:, 15:, :], qt[:, 15:, :])
                state[r] = (q8, sc, al)

            def cons_chunk(r, c, ot):
                """One 512-wide out chunk; ot holds a 1024-wide store pair."""
                q8, sc, al = state[r]
                t0 = r * 128
                o0 = c * CH
                p0 = ps0.tile([128, CH], f32)
                for e in range(KE2):
                    nc.tensor.matmul(
                        p0[:, :], q8[:, 2 * e:2 * e + 2, :],
                        wq_s[:, c, e, :, :],
                        start=(e == 0), stop=(e == KE2 - 1),
                        perf_mode=DR)
                p1 = ps1.tile([128, CH], f32)
                nc.tensor.matmul(
                    p1[:, :], fpx_s[:, :, t0:t0 + 128],
                    fpw_s[:, :, o0:o0 + CH],
                    start=True, stop=True, perf_mode=DR)
                # out = (p0*scale_t)*wscale + alpha_t*reduced_w + fp(p1)
                m = mp.tile([128, CH], f32, tag="m")
                nc.vector.scalar_tensor_tensor(
                    out=m[:, :], in0=p0[:, :], scalar=sc[:, :],
                    in1=wsb_s[:, o0:o0 + CH], op0=Alu.mult, op1=Alu.mult)
                m2 = mp.tile([128, CH], f32, tag="m2")
                nc.vector.scalar_tensor_tensor(
                    out=m2[:, :], in0=rwb_s[:, o0:o0 + CH], scalar=al[:, :],
                    in1=p1[:, :], op0=Alu.mult, op1=Alu.add)
                os = (c % 2) * CH
                nc.vector.tensor_tensor(
                    out=ot[:, os:os + CH], in0=m[:, :], in1=m2[:, :],
                    op=Alu.add)
                if c % 2 == 1:
                    nc.gpsimd.dma_start(
                        out=out_d[t0:t0 + 128, o0 - CH:o0 + CH],
                        in_=ot[:, :])

            def cons_range(r, c_lo, c_hi):
                for cp in range(c_lo // 2, c_hi // 2):
                    ot = outp.tile([128, 2 * CH], f16)
                    cons_chunk(r, 2 * cp, ot)
                    cons_chunk(r, 2 * cp + 1, ot)

            # Software pipeline.  prod_a(r+2) is sandwiched inside
            # consumer(r) so its DVE work (round) doesn't delay the
            # combine ops that free PSUM banks; prod_b(r+1) (transpose+
            # cast) is emitted after cons(r) so qt/q8 double buffers are
            # never overwritten before their consumers are emitted.
            prod_a(0)
            prod_b(0)
            prod_a(1)
            for r in range(NT):
                cons_range(r, 0, NCH // 2)
                if r + 1 < NT:
                    prod_b(r + 1)
                if r + 2 < NT:
                    prod_a(r + 2)
                cons_range(r, NCH // 2, NCH)

    nc.finalize()
    return nc


def _get_program():
    global _PROGRAM
    if _PROGRAM is None:
        _PROGRAM = _build_program()
    return _PROGRAM


def _unpack_i4(w_packed):
    """(out, INT_F//2) uint8 -> (out, INT_F) int8; col 2k=low nibble, 2k+1=high."""
    lo = (w_packed & 0x0F).astype(np.int8)
    hi = ((w_packed >> 4) & 0x0F).astype(np.int8)
    lo = np.where(lo >= 8, lo - 16, lo)
    hi = np.where(hi >= 8, hi - 16, hi)
    w = np.empty((w_packed.shape[0], w_packed.shape[1] * 2), dtype=np.int8)
    w[:, 0::2] = lo
    w[:, 1::2] = hi
    return w


def _prep_inputs(x, int_weight, weights_scales, reduced_w, fp_weight, bias,
                 int_indices, fp_indices):
    import ml_dtypes
    f8np = ml_dtypes.float8_e4m3

    x2 = np.asarray(x, dtype=np.float16)[0]
    int_idx = np.asarray(int_indices).astype(np.int64)
    fp_idx = np.asarray(fp_indices).astype(np.int64)

    x_int = np.ascontiguousarray(x2[:, int_idx])            # (M, 3840) f16
    fp_xT = np.ascontiguousarray(x2[:, fp_idx].T)           # (256, M) f16

    w_int = _unpack_i4(np.asarray(int_weight))              # (OUT_F, 3840) int8
    # wq[c, p, e, r, o] = w_int[(2e+r)*128+p, c*CH+o]
    wq = np.ascontiguousarray(
        w_int.T.reshape(KE2, 2, 128, NCH, CH).transpose(3, 2, 0, 1, 4)
    ).astype(f8np)

    wsc = np.asarray(weights_scales).astype(np.float16)     # (OUT_F, 1)
    wsb = np.ascontiguousarray(
        np.broadcast_to(wsc[:, 0][None, :], (128, OUT_F)))  # (128, OUT_F) f16

    redw = np.asarray(reduced_w).astype(np.float16)         # (1, OUT_F)
    rwb = np.ascontiguousarray(
        np.broadcast_to(redw[0][None, :], (128, OUT_F)))    # (128, OUT_F) f16

    fpW = np.asarray(fp_weight).astype(np.float16)          # (OUT_F, 256)
    fpw = np.ascontiguousarray(
        fpW.T.reshape(2, 128, OUT_F).transpose(1, 0, 2)).astype(f8np)

    in_maps = []
    for c in range(NCORES):
        t0 = c * TPT
        x_c = np.ascontiguousarray(x_int[t0:t0 + TPT])
        fpx = np.ascontiguousarray(
            fp_xT[:, t0:t0 + TPT].reshape(2, 128, TPT).transpose(1, 0, 2)
        ).astype(f8np)
        in_maps.append({"x_st": x_c, "wq": wq, "fpx": fpx, "fpw": fpw,
                        "wsb": wsb, "rwb": rwb})
    return in_maps


def kernel(x, int_weight, weights_scales, reduced_w, fp_weight, bias,
           int_indices, fp_indices):
    global LAST_RESULTS
    from concourse.bass_utils import run_bass_kernel_spmd

    _ensure_ntff_hook()
    in_maps = _prep_inputs(x, int_weight, weights_scales, reduced_w,
                           fp_weight, bias, int_indices, fp_indices)
    nc = _get_program()
    res = run_bass_kernel_spmd(nc, in_maps, core_ids=list(range(NCORES)))
    LAST_RESULTS = res
    out = np.concatenate([res.results[c]["out"] for c in range(NCORES)], axis=0)
    # bias is applied host-side (it no longer rides the device fp matmul)
    out = out.astype(np.float32) + np.asarray(bias).astype(np.float32)[None, :]
    return out[None].astype(np.float16)


# revision 32
# speedup vs baseline: 1.1057x; 1.1057x over previous
"""MixedQLinear Trainium2 kernel (v2: token-sharded).

Computation (per reference):
  x2 = x[0]                                  (M=4096, IN_F=4096) fp16
  int_x = x2[:, int_indices]                 (M, 3840)
  fp_x  = x2[:, fp_indices]                  (M, 256)
  per-token asym quant of int_x to int4:  scale=(mx-mn)/15, zero=mn
  q = round((int_x-zero)/scale) - 8          in [-8,7]
  out = scale*w_scale*(q @ w_int.T) + (zero+8*scale)*reduced_w + fp_x@fp_w.T + bias

Sharding: TOKENS are sharded 8 ways (512 tokens per core); every core
holds the FULL weight set (int4 weights as fp8 = 15 MiB) and computes
all 4096 out_features for its own tokens.  No collectives.  This makes
the per-token quantization work (stats, scale, round, transpose) 8x
smaller than out-feature sharding, and lets the int matmul run as long
fp8 DoubleRow instructions with ap_size 1024 so the PE stays busy and
ramps to its full clock.

Per core device flow (4 token tiles of 128):
  - stats: min tree (DVE) + max tree (GpSimd) -> scale, rs, bq, alpha
  - quantize: ACT y0 = x*rs+bq (f32, quarters), DVE RNE round via the
    fp32 magic constant -> qa f16 [128, 3968] (cols 3840/3841 = alpha/1)
  - two half DMA xbar transposes -> qt [128, 31, 128] k-major
  - ACT cast to fp8 -> q8 [128, 30, 128]
  - per 1024-col out chunk: 15 fp8 DoubleRow matmuls (q8 stationary,
    weights moving) -> p0; fp8-DR fp-path matmul + f16 extras matmul
    (alpha row x reduced_w, ones x bias) -> p1
  - combine: m = (p0*scale_t)*wscale (scalar_tensor_tensor), out = m+p1

Host side does layout only: column gather, int4 unpack, fp8 casts,
sharding, concat of per-core outputs along tokens.
"""

import os
import sys

import numpy as np

for _p in ("/opt/trn_rl_repo",):
    if _p not in sys.path and os.path.isdir(_p):
        sys.path.insert(0, _p)

TOKENS = 4096
IN_F = 4096
OUT_F = 4096
FP_F = 256
INT_F = IN_F - FP_F          # 3840
NCORES = 8
TPT = TOKENS // NCORES       # 512 own tokens per core
NT = TPT // 128              # 4 own token tiles
KE = INT_F // 128            # 30 k-groups
KE2 = KE // 2                # 15 DoubleRow steps
CH = 512                     # out-feature chunk (1 PSUM bank of f32)
NCH = OUT_F // CH            # 8 chunks
QW = 3968                    # qa width: 3840 q + alpha + ones + pad (31*128)
C_MAGIC = 12582912.0         # 1.5*2^23: fp32 add/sub forces RNE-to-integer

_PROGRAM = None
LAST_RESULTS = None


def _ensure_ntff_hook():
    """Install the axon NTFF profiling hook if the image's antenv lacks it.

    Best-effort: profiling only; compile/run work without it.
    """
    import contextlib
    import ctypes
    import types

    try:
        try:
            import antenv.axon_hooks as hooks_mod
        except ImportError:
            import antenv

            hooks_mod = types.ModuleType("antenv.axon_hooks")
            _holder = {}
            hooks_mod.set_axon_ntff_profile_hook = (
                lambda hook: _holder.__setitem__("hook", hook))
            hooks_mod.get_axon_ntff_profile_hook = (
                lambda: _holder.get("hook"))
            sys.modules["antenv.axon_hooks"] = hooks_mod
            antenv.axon_hooks = hooks_mod

        if hooks_mod.get_axon_ntff_profile_hook() is not None:
            return
        so_path = "/opt/axon/libaxon_pjrt.so"
        if not os.path.exists(so_path):
            return
        lib = ctypes.CDLL(so_path)
        if not hasattr(lib, "axon_start_nrt_profile"):
            return
        lib.axon_start_nrt_profile.argtypes = [
            ctypes.POINTER(ctypes.c_int64), ctypes.c_size_t]
        lib.axon_start_nrt_profile.restype = ctypes.c_int64
        lib.axon_stop_nrt_profile.argtypes = [ctypes.c_char_p]
        lib.axon_stop_nrt_profile.restype = ctypes.c_int64

        @contextlib.contextmanager
        def _hook(output_dir, device_ids):
            import jax

            jax.devices()
            if device_ids:
                ids = (ctypes.c_int64 * len(device_ids))(*device_ids)
                rc = lib.axon_start_nrt_profile(ids, len(device_ids))
            else:
                rc = lib.axon_start_nrt_profile(None, 0)
            if rc != 0:
                raise RuntimeError(f"axon_start_nrt_profile rc={rc}")
            try:
                yield
            finally:
                n = lib.axon_stop_nrt_profile(str(output_dir).encode())
                print(f"ntff profile: {n} file(s) written to {output_dir}")

        hooks_mod.set_axon_ntff_profile_hook(_hook)
    except Exception:
        pass


def _build_program():
    import concourse.mybir as mybir
    import concourse.tile as tile
    from concourse import bacc

    f16 = mybir.dt.float16
    f32 = mybir.dt.float32
    f8 = mybir.dt.float8e4
    Alu = mybir.AluOpType
    DR = mybir.MatmulPerfMode.DoubleRow

    nc = bacc.Bacc(None, target_bir_lowering=False)

    x_st = nc.dram_tensor("x_st", [TPT, INT_F], f16, kind="ExternalInput")
    # int4 weight values (exact in fp8e4m3), chunk-column-major k-major DR
    # layout: wq[c, p, e, r, o] = w_int[(2e+r)*128 + p, c*CH + o]
    wq_d = nc.dram_tensor("wq", [NCH, 128, KE2, 2, CH], f8,
                          kind="ExternalInput")
    fpx_d = nc.dram_tensor("fpx", [128, 2, TPT], f8, kind="ExternalInput")
    fpw_d = nc.dram_tensor("fpw", [128, 2, OUT_F], f8, kind="ExternalInput")
    wsb_d = nc.dram_tensor("wsb", [128, OUT_F], f16, kind="ExternalInput")
    rwb_d = nc.dram_tensor("rwb", [128, OUT_F], f16, kind="ExternalInput")
    out_d = nc.dram_tensor("out", [TPT, OUT_F], f16, kind="ExternalOutput")

    with tile.TileContext(nc) as tc:
        with tc.tile_pool(name="consts", bufs=1) as consts, \
             tc.tile_pool(name="xin", bufs=1) as xin, \
             tc.tile_pool(name="y0p", bufs=2) as y0p, \
             tc.tile_pool(name="qap", bufs=2) as qap, \
             tc.tile_pool(name="qtp", bufs=2) as qtp, \
             tc.tile_pool(name="qt8", bufs=2) as qt8, \
             tc.tile_pool(name="jnk", bufs=1) as jnk, \
             tc.tile_pool(name="stp", bufs=4) as stp, \
             tc.tile_pool(name="mp", bufs=1) as mp, \
             tc.tile_pool(name="outp", bufs=2) as outp, \
             tc.tile_pool(name="ps0", bufs=4, space="PSUM") as ps0, \
             tc.tile_pool(name="ps1", bufs=4, space="PSUM") as ps1:

            # Const loads on the scalar (ACT) HWDGE queue.  Order matters:
            # wq chunk-column c gates the consumer's chunk c, and the small
            # fp/scale tensors are needed from chunk 0 on, so they ride
            # between wq chunks 0 and 1.
            # All DMA transfers serialize on one DMA-engine resource, so
            # only global order matters: weight chunk-columns stream on the
            # scalar queue in consumption order, interleaved with the small
            # fp/scale tensors right when the first combines need them.
            # All DMA serializes on one DMA-engine resource in request order,
            # so the scalar queue carries the demand-ordered bulk stream:
            # x0 (gates the whole producer pipeline), then weights with the
            # combine constants early enough for the first combine.
            wq_s = consts.tile([128, NCH, KE2, 2, CH], f8)
            fpw_s = consts.tile([128, 2, OUT_F], f8)
            wsb_s = consts.tile([128, OUT_F], f16)
            rwb_s = consts.tile([128, OUT_F], f16)
            # own-token fp columns ride the gpsimd queue with the x tiles
            fpx_s = consts.tile([128, 2, TPT], f8)

            state = {}

            def load_consts():
                nc.scalar.dma_start(out=wq_s[:, 0], in_=wq_d[0])
                nc.scalar.dma_start(out=fpw_s[:, :, :], in_=fpw_d[:, :, :])
                nc.scalar.dma_start(out=wsb_s[:, :], in_=wsb_d[:, :])
                nc.scalar.dma_start(out=rwb_s[:, :], in_=rwb_d[:, :])
                for c in range(1, NCH):
                    nc.scalar.dma_start(out=wq_s[:, c], in_=wq_d[c])

            def prod_a(r):
                """x load, stats, quantize, round -> qa."""
                xt = xin.tile([128, INT_F], f16)
                # x0 leads the scalar bulk-stream (it gates everything);
                # later x tiles ride sync, WAR-throttled by xin bufs=1 so
                # they enter the DMA queue just-in-time.
                eng = nc.scalar if r == 0 else nc.sync
                eng.dma_start(
                    out=xt[:, :], in_=x_st[r * 128:(r + 1) * 128, :])
                if r == 0:
                    nc.gpsimd.dma_start(out=fpx_s[:, :, :], in_=fpx_d[:, :, :])
                mn = stp.tile([128, 1], f32, tag="mn")
                mx = stp.tile([128, 1], f32, tag="mx")
                a1 = jnk.tile([128, 1920], f16, tag="a1")
                nc.vector.tensor_tensor(
                    out=a1[:, :], in0=xt[:, :1920], in1=xt[:, 1920:],
                    op=Alu.min)
                nc.vector.tensor_reduce(
                    out=mn[:, :], in_=a1[:, :], axis=mybir.AxisListType.X,
                    op=Alu.min)
                a1b = jnk.tile([128, 1920], f16, tag="a1")
                nc.vector.tensor_tensor(
                    out=a1b[:, :], in0=xt[:, :1920], in1=xt[:, 1920:],
                    op=Alu.max)
                nc.vector.tensor_reduce(
                    out=mx[:, :], in_=a1b[:, :], axis=mybir.AxisListType.X,
                    op=Alu.max)
                # scale = max((mx-mn)/15, 1e-8); rs = 1/scale;
                # bq = -mn*rs - 8; alpha = mn + 8*scale
                d = stp.tile([128, 1], f32, tag="d")
                nc.vector.tensor_sub(d[:, :], mx[:, :], mn[:, :])
                sc = stp.tile([128, 1], f32, tag="sc")
                nc.vector.tensor_scalar(
                    out=sc[:, :], in0=d[:, :],
                    scalar1=1.0 / 15.0, scalar2=1e-8, op0=Alu.mult, op1=Alu.max)
                rs = stp.tile([128, 1], f32, tag="rs")
                nc.vector.reciprocal(rs[:, :], sc[:, :])
                tt = stp.tile([128, 1], f32, tag="tt")
                nc.vector.tensor_mul(tt[:, :], mn[:, :], rs[:, :])
                bq = stp.tile([128, 1], f32, tag="bq")
                nc.vector.tensor_scalar(
                    out=bq[:, :], in0=tt[:, :],
                    scalar1=-1.0, scalar2=-8.0, op0=Alu.mult, op1=Alu.add)
                t8 = stp.tile([128, 1], f32, tag="t8")
                nc.vector.tensor_scalar(
                    out=t8[:, :], in0=sc[:, :],
                    scalar1=8.0, scalar2=None, op0=Alu.mult)
                al = stp.tile([128, 1], f32, tag="al")
                nc.vector.tensor_add(al[:, :], t8[:, :], mn[:, :])
                qa = qap.tile([128, INT_F], f16)
                # quantize+round in eighths: ACT y0=x*rs+bq, DVE RNE round
                for h in range(8):
                    lo, hi = h * 480, (h + 1) * 480
                    y0 = y0p.tile([128, 480], f32, tag="y0")
                    nc.scalar.activation(
                        out=y0[:, :], in_=xt[:, lo:hi],
                        func=mybir.ActivationFunctionType.Identity,
                        bias=bq[:, :], scale=rs[:, :])
                    nc.vector.tensor_scalar(
                        out=qa[:, lo:hi], in0=y0[:, :], scalar1=C_MAGIC,
                        scalar2=-C_MAGIC, op0=Alu.add, op1=Alu.add)
                state[r] = (qa, sc, al)

            def prod_b(r):
                """Transpose qa -> qt (k-major), cast to fp8 -> q8."""
                qa, sc, al = state[r]
                qt = qtp.tile([128, KE, 128], f16)
                nc.sync.dma_start_transpose(
                    out=qt[:, :15, :], in_=qa[:, :1920])
                nc.sync.dma_start_transpose(
                    out=qt[:, 15:, :], in_=qa[:, 1920:])
                q8 = qt8.tile([128, KE, 128], f8)
                nc.scalar.copy(q8[:, :15, :], qt[:, :15, :])
                nc.scalar.copy(q8[:, 15:, :], qt[:, 15:, :])
                state[r] = (q8, sc, al)

            def cons_chunk(r, c, ot):
                """One 512-wide out chunk; ot holds a 1024-wide store pair."""
                q8, sc, al = state[r]
                t0 = r * 128
                o0 = c * CH
                p0 = ps0.tile([128, CH], f32)
                for e in range(KE2):
                    nc.tensor.matmul(
                        p0[:, :], q8[:, 2 * e:2 * e + 2, :],
                        wq_s[:, c, e, :, :],
                        start=(e == 0), stop=(e == KE2 - 1),
                        perf_mode=DR)
                p1 = ps1.tile([128, CH], f32)
                nc.tensor.matmul(
                    p1[:, :], fpx_s[:, :, t0:t0 + 128],
                    fpw_s[:, :, o0:o0 + CH],
                    start=True, stop=True, perf_mode=DR)
                # out = (p0*scale_t)*wscale + alpha_t*reduced_w + fp(p1)
                m = mp.tile([128, CH], f32, tag="m")
                nc.vector.scalar_tensor_tensor(
                    out=m[:, :], in0=p0[:, :], scalar=sc[:, :],
                    in1=wsb_s[:, o0:o0 + CH], op0=Alu.mult, op1=Alu.mult)
                m2 = mp.tile([128, CH], f32, tag="m2")
                nc.vector.scalar_tensor_tensor(
                    out=m2[:, :], in0=rwb_s[:, o0:o0 + CH], scalar=al[:, :],
                    in1=p1[:, :], op0=Alu.mult, op1=Alu.add)
                os = (c % 2) * CH
                nc.vector.tensor_tensor(
                    out=ot[:, os:os + CH], in0=m[:, :], in1=m2[:, :],
                    op=Alu.add)
                if c % 2 == 1:
                    nc.gpsimd.dma_start(
                        out=out_d[t0:t0 + 128, o0 - CH:o0 + CH],
                        in_=ot[:, :])

            def cons_range(r, c_lo, c_hi):
                for cp in range(c_lo // 2, c_hi // 2):
                    ot = outp.tile([128, 2 * CH], f16)
                    cons_chunk(r, 2 * cp, ot)
                    cons_chunk(r, 2 * cp + 1, ot)

            # Software pipeline.  prod_a(r+2) is sandwiched inside
            # consumer(r) so its DVE work (round) doesn't delay the
            # combine ops that free PSUM banks; prod_b(r+1) (transpose+
            # cast) is emitted after cons(r) so qt/q8 double buffers are
            # never overwritten before their consumers are emitted.
            prod_a(0)
            load_consts()
            prod_b(0)
            prod_a(1)
            for r in range(NT):
                cons_range(r, 0, NCH // 2)
                if r + 1 < NT:
                    prod_b(r + 1)
                if r + 2 < NT:
                    prod_a(r + 2)
                cons_range(r, NCH // 2, NCH)

    nc.finalize()
    return nc


def _get_program():
    global _PROGRAM
    if _PROGRAM is None:
        _PROGRAM = _build_program()
    return _PROGRAM


def _unpack_i4(w_packed):
    """(out, INT_F//2) uint8 -> (out, INT_F) int8; col 2k=low nibble, 2k+1=high."""
    lo = (w_packed & 0x0F).astype(np.int8)
    hi = ((w_packed >> 4) & 0x0F).astype(np.int8)
    lo = np.where(lo >= 8, lo - 16, lo)
    hi = np.where(hi >= 8, hi - 16, hi)
    w = np.empty((w_packed.shape[0], w_packed.shape[1] * 2), dtype=np.int8)
    w[:, 0::2] = lo
    w[:, 1::2] = hi
    return w


def _prep_inputs(x, int_weight, weights_scales, reduced_w, fp_weight, bias,
                 int_indices, fp_indices):
    import ml_dtypes
    f8np = ml_dtypes.float8_e4m3

    x2 = np.asarray(x, dtype=np.float16)[0]
    int_idx = np.asarray(int_indices).astype(np.int64)
    fp_idx = np.asarray(fp_indices).astype(np.int64)

    x_int = np.ascontiguousarray(x2[:, int_idx])            # (M, 3840) f16
    fp_xT = np.ascontiguousarray(x2[:, fp_idx].T)           # (256, M) f16

    w_int = _unpack_i4(np.asarray(int_weight))              # (OUT_F, 3840) int8
    # wq[c, p, e, r, o] = w_int[(2e+r)*128+p, c*CH+o]
    wq = np.ascontiguousarray(
        w_int.T.reshape(KE2, 2, 128, NCH, CH).transpose(3, 2, 0, 1, 4)
    ).astype(f8np)

    wsc = np.asarray(weights_scales).astype(np.float16)     # (OUT_F, 1)
    wsb = np.ascontiguousarray(
        np.broadcast_to(wsc[:, 0][None, :], (128, OUT_F)))  # (128, OUT_F) f16

    redw = np.asarray(reduced_w).astype(np.float16)         # (1, OUT_F)
    rwb = np.ascontiguousarray(
        np.broadcast_to(redw[0][None, :], (128, OUT_F)))    # (128, OUT_F) f16

    fpW = np.asarray(fp_weight).astype(np.float16)          # (OUT_F, 256)
    fpw = np.ascontiguousarray(
        fpW.T.reshape(2, 128, OUT_F).transpose(1, 0, 2)).astype(f8np)

    in_maps = []
    for c in range(NCORES):
        t0 = c * TPT
        x_c = np.ascontiguousarray(x_int[t0:t0 + TPT])
        fpx = np.ascontiguousarray(
            fp_xT[:, t0:t0 + TPT].reshape(2, 128, TPT).transpose(1, 0, 2)
        ).astype(f8np)
        in_maps.append({"x_st": x_c, "wq": wq, "fpx": fpx, "fpw": fpw,
                        "wsb": wsb, "rwb": rwb})
    return in_maps


def kernel(x, int_weight, weights_scales, reduced_w, fp_weight, bias,
           int_indices, fp_indices):
    global LAST_RESULTS
    from concourse.bass_utils import run_bass_kernel_spmd

    _ensure_ntff_hook()
    in_maps = _prep_inputs(x, int_weight, weights_scales, reduced_w,
                           fp_weight, bias, int_indices, fp_indices)
    nc = _get_program()
    res = run_bass_kernel_spmd(nc, in_maps, core_ids=list(range(NCORES)))
    LAST_RESULTS = res
    out = np.concatenate([res.results[c]["out"] for c in range(NCORES)], axis=0)
    # bias is applied host-side (it no longer rides the device fp matmul)
    out = out.astype(np.float32) + np.asarray(bias).astype(np.float32)[None, :]
    return out[None].astype(np.float16)
